# revision 6
# baseline (speedup 1.0000x reference)
"""AttnVLAD layer on 8 Trainium2 NeuronCores.

Data-parallel over batch: b=32 samples -> 4 per core. Host precomputes
fp16 copies of x in both layouts (d-major for mm1, n-major for mm2) plus
the fp16 split of q = alpha * centers/||centers||, so the device does no
casting or transposing of x. The global L2 normalize is folded into the
cluster weights on the host (rows are unit-normed, so the global norm is
||cw||_2 exactly). Per sample:
  scoreT[n,K] = qh^T xh             (fp16 matmuls, fp32 PSUM accum)
  prob = softmax over K (fp16)      (one exp per 1024-n bank)
  descT[K,d] = prob^T @ xT          (fp16 matmuls, fp32 PSUM accum)
  denomT[K,1] rides mm2's stationary (prob^T @ ones)
  epilogue in [K,D] layout: denom-normalize, subtract centersT,
  intra-L2, weighted by cw/||cw|| -> out[K,D] (host transposes back)

x streams in 1MB slabs (1024 n each) through deep rings so the 16 DMA
engines never starve; DMA dispatch is split between the SP and ACT
HWDGE queues to keep either sequencer off the critical path.
"""
import numpy as np

B, D, N, K = 32, 512, 4096, 64
NCORES = 8
SPC = B // NCORES          # samples per core
DCH = D // 128             # 4 d-chunks
NCH = N // 128             # 32 n-chunks
BPB = 8                    # n-chunks per bank/slab
NB = NCH // BPB            # 4 banks per sample
NQN = 128 * BPB            # 1024 n per slab
NG = SPC * NB              # 16 global banks per core
PRE = 5                    # slab-pairs prefetched ahead
RING_XH = 6
RING_XT = 8

_COMPILED = {}


def _build():
    import concourse.bass as bass
    import concourse.bacc as bacc
    import concourse.tile as tile
    import concourse.mybir as mybir

    f32 = mybir.dt.float32
    f16 = mybir.dt.float16
    AF = mybir.ActivationFunctionType
    OP = mybir.AluOpType
    AX = mybir.AxisListType

    nc = bacc.Bacc("TRN2", target_bir_lowering=False, debug=False)
    xh_dram = nc.dram_tensor("xh", [SPC, D, N], f16, kind="ExternalInput")
    xT_dram = nc.dram_tensor("xT", [SPC, N, D], f16, kind="ExternalInput")
    qh_dram = nc.dram_tensor("qh", [D, K], f16, kind="ExternalInput")
    cT_dram = nc.dram_tensor("cT", [K, D], f32, kind="ExternalInput")
    cw_dram = nc.dram_tensor("cw", [K, 1], f32, kind="ExternalInput")
    out_dram = nc.dram_tensor("out", [SPC, K, D], f32, kind="ExternalOutput")

    with tile.TileContext(nc) as tc:
        with (
            tc.tile_pool(name="const", bufs=1) as const,
            tc.tile_pool(name="xhp", bufs=RING_XH) as xhp,
            tc.tile_pool(name="xTp", bufs=RING_XT) as xTp,
            tc.tile_pool(name="probp", bufs=2) as probp,
            tc.tile_pool(name="s16p", bufs=2) as s16p,
            tc.tile_pool(name="e16p", bufs=2) as e16p,
            tc.tile_pool(name="smp", bufs=4) as smp,
            tc.tile_pool(name="epp", bufs=1) as epp,
            tc.tile_pool(name="ps_sc", bufs=4, space="PSUM") as ps_sc,
            tc.tile_pool(name="ps_d", bufs=2, space="PSUM") as ps_d,
            tc.tile_pool(name="ps_n", bufs=2, space="PSUM") as ps_n,
        ):
            # ---------- slab loaders: xh on SP queue, xT on ACT queue ----
            xh_slabs = {}
            xT_slabs = {}

            def load_slab(g):
                s, b = g >> 2, g & 3
                xh = xhp.tile([128, DCH, NQN], f16, tag="xh", name=f"xh{g}")
                xT = xTp.tile([128, BPB, D], f16, tag="xT", name=f"xT{g}")
                nc.sync.dma_start(
                    xh[:], xh_dram[s, :, b * NQN:(b + 1) * NQN]
                    .rearrange("(c p) n -> p c n", p=128))
                nc.sync.dma_start(
                    xT[:], xT_dram[s, b * NQN:(b + 1) * NQN, :]
                    .rearrange("(j p) d -> p j d", p=128))
                xh_slabs[g] = xh
                xT_slabs[g] = xT

            # kick off the first slab before anything else queues
            load_slab(0)

            qh_sb = const.tile([128, DCH, K], f16, tag="qh_sb")
            nc.sync.dma_start(
                qh_sb[:], qh_dram[:].rearrange("(c p) k -> p c k", p=128))
            ones16 = const.tile([128, 1], f16, tag="ones16")
            nc.gpsimd.memset(ones16[:], 1.0)

            for g in range(1, PRE):
                load_slab(g)
            cT_sb = const.tile([K, D], f32, tag="cT_sb")
            nc.sync.dma_start(cT_sb[:], cT_dram[:])
            cw_sb = const.tile([K, 1], f32, tag="cw_sb")
            nc.sync.dma_start(cw_sb[:], cw_dram[:])

            probs = {}   # per-sample probs tiles
            descT = {}   # per-sample desc psum
            denomT = {}  # per-sample denom psum
            pending = []  # deferred epilogues

            def mm1_bank(g):
                xh = xh_slabs[g]
                bank = ps_sc.tile([128, BPB, K], f32, tag="scoreT",
                                  name=f"scb{g}")
                first = True
                for dc in range(DCH):
                    for c in range(BPB):
                        last = (dc == DCH - 1 and c == BPB - 1)
                        nc.tensor.matmul(
                            bank[:, c, :],
                            xh[:, dc, c * 128:(c + 1) * 128],
                            qh_sb[:, dc, :],
                            start=first, stop=last,
                            skip_group_check=(not first))
                        first = False
                return bank

            def softmax_bank(g, bank):
                s, b = g >> 2, g & 3
                negmax = smp.tile([128, BPB], f32, tag="negmax")
                nc.vector.reduce_max(negmax[:].unsqueeze(2), bank[:],
                                     axis=AX.X, negate=True)
                s16 = s16p.tile([128, BPB, K], f16, tag="s16", name=f"s{g}")
                nc.vector.tensor_add(
                    s16[:], bank[:],
                    negmax[:].unsqueeze(2).broadcast_to([128, BPB, K]))
                e16 = e16p.tile([128, BPB, K], f16, tag="e16", name=f"e{g}")
                nc.scalar.activation(e16[:], s16[:], AF.Exp)
                rs = smp.tile([128, BPB], f32, tag="rs")
                nc.vector.reduce_sum(rs[:].unsqueeze(2), e16[:], axis=AX.X)
                rr = smp.tile([128, BPB], f32, tag="rr")
                nc.vector.reciprocal(rr[:], rs[:])
                rr16 = smp.tile([128, BPB], f16, tag="rr16")
                nc.vector.tensor_copy(rr16[:], rr[:])
                nc.vector.tensor_mul(
                    probs[s][:, b * BPB:(b + 1) * BPB, :], e16[:],
                    rr16[:].unsqueeze(2).broadcast_to([128, BPB, K]))

            def mm2_bank(g):
                s, b = g >> 2, g & 3
                xT = xT_slabs[g]
                pr = probs[s]
                for c in range(BPB):
                    j = b * BPB + c
                    nc.tensor.matmul(descT[s][:], pr[:, j, :], xT[:, c, :],
                                     start=(j == 0), stop=(j == NCH - 1),
                                     skip_group_check=(j != 0))
                    nc.tensor.matmul(denomT[s][:], pr[:, j, :], ones16[:],
                                     start=(j == 0), stop=(j == NCH - 1),
                                     skip_group_check=(j != 0))

            def make_epilogue(s, dT, dn):
                def run():
                    rdenom = epp.tile([K, 1], f32, tag="rdenom",
                                      name=f"rd{s}")
                    nc.vector.tensor_scalar_max(rdenom[:], dn[:], 1e-6)
                    nc.vector.reciprocal(rdenom[:], rdenom[:])
                    desc_c = epp.tile([K, D], f32, tag="desc_c",
                                      name=f"dcc{s}")
                    nc.vector.scalar_tensor_tensor(
                        desc_c[:], in0=dT[:], scalar=rdenom[:],
                        in1=cT_sb[:], op0=OP.mult, op1=OP.subtract)
                    sqe = epp.tile([K, D], f32, tag="sqe", name=f"sq{s}")
                    ss = epp.tile([K, 1], f32, tag="ss", name=f"ss{s}")
                    nc.vector.tensor_mul(sqe[:], desc_c[:], desc_c[:])
                    nc.vector.reduce_sum(ss[:], sqe[:], axis=AX.X)
                    intra = epp.tile([K, 1], f32, tag="intra", name=f"in{s}")
                    nc.scalar.activation(intra[:], ss[:], AF.Sqrt)
                    nc.vector.tensor_scalar_max(intra[:], intra[:], 1e-12)
                    rintra = epp.tile([K, 1], f32, tag="rintra",
                                      name=f"ri{s}")
                    nc.vector.reciprocal(rintra[:], intra[:])
                    sfin = epp.tile([K, 1], f32, tag="sfin", name=f"sf{s}")
                    nc.vector.tensor_mul(sfin[:], cw_sb[:], rintra[:])
                    outT = epp.tile([K, D], f32, tag="outT", name=f"oT{s}")
                    nc.vector.tensor_mul(outT[:], desc_c[:],
                                         sfin[:].broadcast_to([K, D]))
                    nc.sync.dma_start(out_dram[s], outT[:])
                return run

            for g in range(NG):
                s, b = g >> 2, g & 3
                if b == 0:
                    probs[s] = probp.tile([128, NCH, K], f16, tag="prob",
                                          name=f"pr{s}")
                bank = mm1_bank(g)
                softmax_bank(g, bank)
                if g >= 2:
                    g2 = g - 2
                    s2, b2 = g2 >> 2, g2 & 3
                    if b2 == 0:
                        descT[s2] = ps_d.tile([K, D], f32, tag="descT",
                                              name=f"dT{s2}")
                        denomT[s2] = ps_n.tile([K, 1], f32, tag="denomT",
                                               name=f"dn{s2}")
                    mm2_bank(g2)
                    if b2 == NB - 1:
                        pending.append(
                            make_epilogue(s2, descT[s2], denomT[s2]))
                if b == 2 and pending:
                    pending.pop(0)()
                if g + PRE < NG:
                    load_slab(g + PRE)

            # drain: mm2 for the last two banks, then remaining epilogues
            for g2 in range(NG - 2, NG):
                s2, b2 = g2 >> 2, g2 & 3
                if b2 == 0:
                    descT[s2] = ps_d.tile([K, D], f32, tag="descT",
                                          name=f"dT{s2}")
                    denomT[s2] = ps_n.tile([K, 1], f32, tag="denomT",
                                           name=f"dn{s2}")
                mm2_bank(g2)
                if b2 == NB - 1:
                    pending.append(make_epilogue(s2, descT[s2], denomT[s2]))
            for fn in pending:
                fn()
            pending.clear()

    nc.compile()
    return nc


def kernel(x, centers, alpha, cluster_weights):
    import concourse.bass_utils as bass_utils

    if "nc" not in _COMPILED:
        _COMPILED["nc"] = _build()
    nc = _COMPILED["nc"]

    x = np.asarray(x, dtype=np.float32)
    xh = np.ascontiguousarray(x.astype(np.float16))
    xT = np.ascontiguousarray(xh.transpose(0, 2, 1))

    c = np.asarray(centers, dtype=np.float64).reshape(D, K)
    a = float(np.asarray(alpha, dtype=np.float64))
    nrm = np.sqrt((c * c).sum(axis=0, keepdims=True))
    q = a * c / np.maximum(nrm, 1e-12)
    qh = q.astype(np.float16)
    cT = np.ascontiguousarray(c.T.astype(np.float32))
    cw = np.asarray(cluster_weights, dtype=np.float64).reshape(K, 1)
    # rows of desc are unit-L2 then scaled by cw, so the flattened norm
    # is ||cw||_2 exactly: fold the final normalize into cw.
    cw_eff = (cw / max(np.sqrt((cw * cw).sum()), 1e-12)).astype(np.float32)

    in_maps = []
    for core in range(NCORES):
        in_maps.append({
            "xh": xh[core * SPC:(core + 1) * SPC],
            "xT": xT[core * SPC:(core + 1) * SPC],
            "qh": qh,
            "cT": cT,
            "cw": cw_eff,
        })
    res = bass_utils.run_bass_kernel_spmd(nc, in_maps,
                                          core_ids=list(range(NCORES)))
    out = np.concatenate([res.results[i]["out"] for i in range(NCORES)],
                         axis=0)                       # [B, K, D]
    return np.ascontiguousarray(
        out.transpose(0, 2, 1).reshape(B, D * K)).astype(np.float32)
